# revision 1
# baseline (speedup 1.0000x reference)
"""HGT layer kernel for 8 Trainium2 NeuronCores.

Strategy: nodes are relabeled so node types are contiguous (padded to 128);
edges are sorted by destination and packed into fixed-size per-128-dst-node
tiles (P=128 partitions x B slabs).  Each core owns a contiguous range of
dst tiles, so segment softmax and aggregation are core-local (no
collectives).  On device each core:
  1. prologue: builds (a) kv_tab[n] = x[n] @ blkdiag(W_K)|blkdiag(W_V)
     (fp32, k||v interleaved, all nodes) and (b) qt_tab[et, own dst] =
     x[dst] @ blkdiag(W_Q W_e^T mu/sqrt(hd)) (bf16, own dst slab only),
  2. per tile: one [P,1]-offset indirect DMA per slab gathers kv rows by
     src (the only random-access traffic); qt rows for the tile's own 128
     dst nodes are loaded contiguously and selected per edge slot with
     one-hot matmuls on the PE (bf16); softmax denominators and message
     aggregation also via one-hot matmuls;
  3. W_out + residual/LayerNorm + FFN + LayerNorm, all fused per tile.
Host only preprocesses integer index arrays and re-assembles the output.

NB: indirect_dma_start is only correct with ONE offset per partition
(ap=[P,1]); multi-offset forms silently gather wrong rows on this build.
"""

import math
import numpy as np

import bass_rust
import concourse.mybir as mybir
import concourse.bass as bass
from concourse.tile import TileContext
from concourse.masks import make_identity
from concourse.bass_utils import run_bass_kernel_spmd

N_NODES = 50000
N_EDGES = 800000
D = 128
H = 8
HD = 16
NT = 3
NET = 5
LN_EPS = 1e-5
NCORES = 8
P = 128
F32 = mybir.dt.float32
BF16 = mybir.dt.bfloat16
F16 = mybir.dt.float16
I32 = mybir.dt.int32


def _bc_ap(ap, ins_pos, n):
    """Insert a stride-0 axis of size n at position ins_pos of an AP."""
    dims = list(ap.ap)
    dims.insert(ins_pos, [0, n])
    return bass.AP(tensor=ap.tensor, offset=ap.offset, ap=dims)


def _build(nc, chunk_types, n_pad, T, B, has_ln_gb1,
           has_ln_gb2, has_bout, has_b1, has_b2):
    ownN = T * P
    xt_bf = nc.dram_tensor("xt_bf", [n_pad // P, D, P], BF16,
                           kind="ExternalInput")
    xt_own = nc.dram_tensor("xt_own", [T, D, P], BF16, kind="ExternalInput")
    x_own = nc.dram_tensor("x_own", [ownN, D], F32, kind="ExternalInput")
    wkv = nc.dram_tensor("wkv", [NT, D, 2 * D], BF16, kind="ExternalInput")
    wqe_own = nc.dram_tensor("wqe_own", [T, D, NET * D], BF16,
                             kind="ExternalInput")
    wout = nc.dram_tensor("wout", [D, D], F32, kind="ExternalInput")
    w1 = nc.dram_tensor("w1", [D, 4 * D], F32, kind="ExternalInput")
    w2 = nc.dram_tensor("w2", [4 * D, D], F32, kind="ExternalInput")
    kvidx = nc.dram_tensor("kvidx", [T, P, B], I32, kind="ExternalInput")
    codes = nc.dram_tensor("codes", [T, P, B], F16, kind="ExternalInput")
    codes5 = nc.dram_tensor("codes5", [T, B * P], F16, kind="ExternalInput")
    iota_f = nc.dram_tensor("iota_f", [P, P], F16, kind="ExternalInput")
    iota5_f = nc.dram_tensor("iota5_f", [P, NET], F16, kind="ExternalInput")
    gb = nc.dram_tensor("gb", [6, P, 4 * D], F32, kind="ExternalInput")
    out = nc.dram_tensor("out", [ownN, D], F32, kind="ExternalOutput")

    kv_tab = nc.dram_tensor("kv_tab", [n_pad, 2 * D], BF16)
    qt_tab = nc.dram_tensor("qt_tab", [NET, ownN, D], BF16)

    with TileContext(nc) as tc:
        with (
            tc.tile_pool(name="const", bufs=1) as const,
            tc.tile_pool(name="wpool", bufs=1) as wpool,
        ):
            ident = const.tile([P, P], F32)
            make_identity(nc, ident)
            iota_sb = const.tile([P, P], F16)
            nc.sync.dma_start(out=iota_sb, in_=iota_f[:, :])
            iota5_sb = const.tile([P, NET], F16)
            nc.sync.dma_start(out=iota5_sb, in_=iota5_f[:, :])
            eps_sb = const.tile([P, 1], F32)
            nc.vector.memset(eps_sb, LN_EPS)
            wkv_sb = wpool.tile([P, NT, 2 * D], BF16)
            nc.sync.dma_start(out=wkv_sb, in_=wkv.rearrange("t d e -> d t e"))
            wout_sb = wpool.tile([P, D], F32)
            nc.sync.dma_start(out=wout_sb, in_=wout[:, :])
            w1_sb = wpool.tile([P, 4 * D], F32)
            nc.sync.dma_start(out=w1_sb, in_=w1[:, :])
            w2_sb = wpool.tile([P, 4, D], F32)
            nc.sync.dma_start(out=w2_sb, in_=w2.rearrange("(c p) d -> p c d", p=P))
            gb_sb = wpool.tile([P, 6, 4 * D], F32)
            nc.sync.dma_start(out=gb_sb, in_=gb.rearrange("g p d -> p g d"))

            # ---------------- prologue: build kv/qt tables ----------------
            # pro stays open through the tile phase: its SBUF is never
            # reused, so tile-phase pools carry no WAR deps on the prologue
            # and the kv gathers can start the moment kv_tab is written.
            with tc.tile_pool(name="pro", bufs=4) as pro:
              with tc.tile_pool(name="pro_ps", bufs=2, space="PSUM") as pro_ps:
                # qt first (its tail overlaps the gather phase), then kv
                # (which gates the gathers).
                QB = 2
                for i0 in range(0, T, QB):
                    nb = min(QB, T - i0)
                    xf = pro.tile([P, QB, P], BF16, tag="xfo")
                    nc.sync.dma_start(
                        out=xf[:, 0:nb, :],
                        in_=xt_own[i0:i0 + nb].rearrange("c d p -> d c p"))
                    wq_sb = pro.tile([P, QB, NET * D], BF16, tag="wq")
                    nc.sync.dma_start(
                        out=wq_sb[:, 0:nb, :],
                        in_=wqe_own[i0:i0 + nb].rearrange("c d e -> d c e"))
                    qt_sb = pro.tile([P, QB, NET * D], BF16, tag="qts")
                    for i in range(nb):
                        qt1_ps = pro_ps.tile([P, 4 * D], F32, tag="q1")
                        nc.tensor.matmul(out=qt1_ps, lhsT=xf[:, i, :],
                                         rhs=wq_sb[:, i, 0:4 * D],
                                         start=True, stop=True)
                        qt2_ps = pro_ps.tile([P, D], F32, tag="q2")
                        nc.tensor.matmul(out=qt2_ps, lhsT=xf[:, i, :],
                                         rhs=wq_sb[:, i, 4 * D:NET * D],
                                         start=True, stop=True)
                        nc.scalar.copy(out=qt_sb[:, i, 0:4 * D], in_=qt1_ps)
                        nc.scalar.copy(out=qt_sb[:, i, 4 * D:], in_=qt2_ps)
                    for e in range(NET):
                        nc.scalar.dma_start(
                            out=qt_tab[e, i0 * P:(i0 + nb) * P, :].rearrange(
                                "(i p) f -> p i f", p=P),
                            in_=qt_sb[:, 0:nb, e * D:(e + 1) * D])
                # kv for ALL nodes, 4 chunks per DMA (sync issues loads,
                # scalar's qActDynamicHW ring issues the table writes)
                KB = 4
                for c0 in range(0, len(chunk_types), KB):
                    nb = min(KB, len(chunk_types) - c0)
                    xf = pro.tile([P, KB, P], BF16, tag="xft")
                    nc.sync.dma_start(
                        out=xf[:, 0:nb, :],
                        in_=xt_bf[c0:c0 + nb].rearrange("c d p -> d c p"))
                    kv_ps = pro_ps.tile([P, KB, 2 * D], F32, tag="kv")
                    for i in range(nb):
                        nc.tensor.matmul(out=kv_ps[:, i, :], lhsT=xf[:, i, :],
                                         rhs=wkv_sb[:, chunk_types[c0 + i], :],
                                         start=True, stop=True)
                    kv_sb = pro.tile([P, KB, 2 * D], BF16, tag="kvs")
                    nc.vector.tensor_copy(out=kv_sb[:, 0:nb, :],
                                          in_=kv_ps[:, 0:nb, :])
                    nc.scalar.dma_start(
                        out=kv_tab[c0 * P:(c0 + nb) * P, :].rearrange(
                            "(c p) e -> p c e", p=P),
                        in_=kv_sb[:, 0:nb, :])

            # ---------------- edge tiles + fused FFN ----------------
              with (
                  tc.tile_pool(name="idx", bufs=6) as idxp,
                  tc.tile_pool(name="edge", bufs=3) as edge,
                  tc.tile_pool(name="mt5p", bufs=2) as mt5p,
                  tc.tile_pool(name="kvp", bufs=4) as kvp,
                  tc.tile_pool(name="small", bufs=3) as small,
                  tc.tile_pool(name="slab", bufs=3) as slab,
                  tc.tile_pool(name="ffn", bufs=2) as ffn,
                  tc.tile_pool(name="qtsel_ps", bufs=2, space="PSUM") as qtsel_psp,
                  tc.tile_pool(name="agg_ps", bufs=2, space="PSUM") as agg_psp,
                  tc.tile_pool(name="ffn_ps", bufs=1, space="PSUM") as ffn_ps,
              ):
                  for t in range(T):
                      kvidx_sb = idxp.tile([P, B], I32, tag="ki")
                      nc.sync.dma_start(out=kvidx_sb, in_=kvidx[t])
                      codes_sb = idxp.tile([P, B], F16, tag="co")
                      nc.sync.dma_start(out=codes_sb, in_=codes[t])
                      # codes5 row broadcast across all partitions
                      c5rep = edge.tile([P, B * P], F16, tag="c5")
                      nc.sync.dma_start(out=c5rep, in_=_bc_ap(codes5[t], 0, P))
                      qt_tile = edge.tile([P, NET, D], BF16, tag="qt")
                      nc.sync.dma_start(
                          out=qt_tile,
                          in_=qt_tab[:, t * P:(t + 1) * P, :].rearrange(
                              "e p f -> p e f"))
                      kv_g = kvp.tile([P, B, 2 * D], BF16, tag="kg")
                      for b in range(B):
                          nc.gpsimd.indirect_dma_start(
                              out=kv_g[:, b, :], out_offset=None,
                              in_=kv_tab[:, :],
                              in_offset=bass.IndirectOffsetOnAxis(
                                  ap=kvidx_sb[:, b:b + 1], axis=0))

                      # one-hot (dst-code) for aggregation: mt[p, b, j]
                      mt = edge.tile([P, B, P], BF16, tag="mt")
                      nc.vector.tensor_tensor(
                          out=mt,
                          in0=codes_sb[:, :].to_broadcast([P, B, P]),
                          in1=_bc_ap(iota_sb[:, :], 1, B),
                          op=mybir.AluOpType.is_equal)
                      # one-hot (128*et + dst-code), transposed: mt5[j, b, e, p]
                      mt5 = mt5p.tile([P, B, NET, P], BF16, tag="mt5")
                      c5_ap = c5rep[:, :].rearrange("j (b p) -> j b p", b=B)
                      nc.vector.tensor_tensor(
                          out=mt5,
                          in0=_bc_ap(c5_ap, 2, NET),
                          in1=_bc_ap(_bc_ap(iota5_sb[:, :], 1, B), 3, P),
                          op=mybir.AluOpType.is_equal)

                      agg_ps = agg_psp.tile([P, P + H], F32)
                      for b in range(B):
                          qtsel_ps = qtsel_psp.tile([P, D], F32)
                          for e in range(NET):
                              nc.tensor.matmul(out=qtsel_ps,
                                               lhsT=mt5[:, b, e, :],
                                               rhs=qt_tile[:, e, :],
                                               start=(e == 0), stop=(e == NET - 1))
                          prod = slab.tile([P, D], F32, tag="pr")
                          nc.vector.tensor_mul(out=prod, in0=qtsel_ps,
                                               in1=kv_g[:, b, 0:D])
                          s_sb = slab.tile([P, H], F32, tag="s")
                          nc.vector.reduce_sum(
                              out=s_sb,
                              in_=prod[:, :].rearrange("p (h f) -> p h f", h=H),
                              axis=mybir.AxisListType.X)
                          ex = slab.tile([P, H], F32, tag="ex")
                          nc.scalar.activation(
                              out=ex, in_=s_sb,
                              func=mybir.ActivationFunctionType.Exp)
                          rhs = slab.tile([P, P + H], BF16, tag="rhs")
                          nc.vector.tensor_copy(out=rhs[:, P:P + H], in_=ex)
                          nc.vector.tensor_tensor(
                              out=rhs[:, 0:P].rearrange("p (h f) -> p h f", h=H),
                              in0=kv_g[:, b, D:2 * D].rearrange(
                                  "p (h f) -> p h f", h=H),
                              in1=ex[:, :].to_broadcast([P, H, HD]),
                              op=mybir.AluOpType.mult)
                          nc.tensor.matmul(out=agg_ps, lhsT=mt[:, b, :],
                                           rhs=rhs,
                                           start=(b == 0), stop=(b == B - 1))

                      den = small.tile([P, H], F32, tag="den")
                      nc.vector.tensor_scalar_add(out=den, in0=agg_ps[:, P:P + H],
                                                  scalar1=1e-10)
                      rcp = small.tile([P, H], F32, tag="rcp")
                      nc.vector.reciprocal(out=rcp, in_=den)
                      aggn = ffn.tile([P, D], F32, tag="aggn")
                      nc.vector.tensor_tensor(
                          out=aggn[:, :].rearrange("p (h f) -> p h f", h=H),
                          in0=agg_ps[:, 0:P].rearrange("p (h f) -> p h f", h=H),
                          in1=rcp[:, :].to_broadcast([P, H, HD]),
                          op=mybir.AluOpType.mult)

                      # ---- W_out + residual + LN1 ----
                      aggn_f_ps = ffn_ps.tile([P, P], F32, tag="tp")
                      nc.tensor.transpose(out=aggn_f_ps, in_=aggn, identity=ident)
                      aggn_f = ffn.tile([P, P], F32, tag="aggnf")
                      nc.scalar.copy(out=aggn_f, in_=aggn_f_ps)
                      mh_ps = ffn_ps.tile([P, D], F32, tag="mh")
                      nc.tensor.matmul(out=mh_ps, lhsT=aggn_f, rhs=wout_sb,
                                       start=True, stop=True)
                      x_sb = ffn.tile([P, D], F32, tag="xo")
                      nc.sync.dma_start(out=x_sb, in_=x_own[t * P:(t + 1) * P, :])
                      h1p = ffn.tile([P, D], F32, tag="h1p")
                      nc.vector.tensor_add(out=h1p, in0=x_sb, in1=mh_ps)
                      if has_bout:
                          nc.vector.tensor_add(out=h1p, in0=h1p, in1=gb_sb[:, 4, 0:D])
                      h1 = _layer_norm(nc, small, ffn, h1p, eps_sb, gb_sb, 0, 1,
                                       has_ln_gb1, "h1")
                      # ---- FFN ----
                      h1f_ps = ffn_ps.tile([P, P], F32, tag="tp")
                      nc.tensor.transpose(out=h1f_ps, in_=h1, identity=ident)
                      h1f = ffn.tile([P, P], F32, tag="h1f")
                      nc.scalar.copy(out=h1f, in_=h1f_ps)
                      p1_ps = ffn_ps.tile([P, 4 * D], F32, tag="p1")
                      nc.tensor.matmul(out=p1_ps, lhsT=h1f, rhs=w1_sb,
                                       start=True, stop=True)
                      g_sb = ffn.tile([P, 4 * D], F32, tag="g")
                      h1v = ffn.tile([P, 4 * D], F32, tag="h1v")
                      if has_b1:
                          nc.vector.tensor_add(out=h1v, in0=p1_ps, in1=gb_sb[:, 5, :])
                      else:
                          nc.scalar.copy(out=h1v, in_=p1_ps)
                      # exact gelu: x * 0.5 * (1 + erf(x/sqrt(2)))
                      nc.scalar.activation(
                          out=g_sb, in_=h1v,
                          func=mybir.ActivationFunctionType.Erf,
                          scale=float(1.0 / math.sqrt(2.0)))
                      nc.vector.tensor_scalar_add(out=g_sb, in0=g_sb, scalar1=1.0)
                      nc.vector.tensor_mul(out=g_sb, in0=g_sb, in1=h1v)
                      nc.vector.tensor_scalar_mul(out=g_sb, in0=g_sb, scalar1=0.5)
                      p2_ps = ffn_ps.tile([P, D], F32, tag="p2")
                      for cc in range(4):
                          gf_ps = ffn_ps.tile([P, P], F32, tag="tp")
                          nc.tensor.transpose(out=gf_ps,
                                              in_=g_sb[:, cc * P:(cc + 1) * P],
                                              identity=ident)
                          gf = ffn.tile([P, P], F32, tag="gf")
                          nc.scalar.copy(out=gf, in_=gf_ps)
                          nc.tensor.matmul(out=p2_ps, lhsT=gf, rhs=w2_sb[:, cc, :],
                                           start=(cc == 0), stop=(cc == 3))
                      o1 = ffn.tile([P, D], F32, tag="o1")
                      nc.vector.tensor_add(out=o1, in0=h1, in1=p2_ps)
                      if has_b2:
                          nc.vector.tensor_add(out=o1, in0=o1, in1=gb_sb[:, 4, D:2 * D])
                      o2 = _layer_norm(nc, small, ffn, o1, eps_sb, gb_sb, 2, 3,
                                       has_ln_gb2, "o2")
                      nc.sync.dma_start(out=out[t * P:(t + 1) * P, :], in_=o2)
    return nc


def _layer_norm(nc, small, ffn, xin, eps_sb, gb_sb, gi, bi, has_gb, tag):
    stats = small.tile([P, 6], F32, tag=tag + "st")
    nc.vector.bn_stats(out=stats, in_=xin)
    mv = small.tile([P, 2], F32, tag=tag + "mv")
    nc.vector.bn_aggr(out=mv, in_=stats)
    sd = small.tile([P, 1], F32, tag=tag + "sd")
    nc.scalar.activation(out=sd, in_=mv[:, 1:2],
                         func=mybir.ActivationFunctionType.Sqrt,
                         bias=eps_sb)
    rs = small.tile([P, 1], F32, tag=tag + "rs")
    nc.vector.reciprocal(out=rs, in_=sd)
    nmb = small.tile([P, 1], F32, tag=tag + "nm")
    nc.vector.tensor_mul(out=nmb, in0=mv[:, 0:1], in1=rs)
    nc.vector.tensor_scalar_mul(out=nmb, in0=nmb, scalar1=-1.0)
    h = ffn.tile([P, D], F32, tag=tag + "h")
    nc.scalar.activation(out=h, in_=xin,
                         func=mybir.ActivationFunctionType.Identity,
                         bias=nmb, scale=rs)
    if has_gb:
        nc.vector.tensor_mul(out=h, in0=h, in1=gb_sb[:, gi, 0:D])
        nc.vector.tensor_add(out=h, in0=h, in1=gb_sb[:, bi, 0:D])
    return h


_CACHE = {}


def kernel(x, edge_index, edge_type, node_type,
           W_Q, W_K, W_V, W_edge, mu,
           W_out, b_out, ln1_g, ln1_b, W1, b1, W2, b2, ln2_g, ln2_b):
    x = np.asarray(x, np.float32)
    src = np.asarray(edge_index[0], np.int64)
    dst = np.asarray(edge_index[1], np.int64)
    et = np.asarray(edge_type, np.int64)
    nt = np.asarray(node_type, np.int64)
    N = x.shape[0]
    E = src.shape[0]

    # ---- node relabeling: group by type, pad each group to 128; within
    # each group, deal nodes round-robin by in-degree so per-tile edge
    # counts are balanced (minimizes B = max slabs per tile) ----
    import heapq
    indeg = np.bincount(dst, minlength=N)
    new_id = np.zeros(N, np.int64)
    base = 0
    chunk_types = []
    for t in range(NT):
        origs = np.where(nt == t)[0]
        ntiles = int(math.ceil(len(origs) / P))
        order_t = origs[np.argsort(-indeg[origs], kind="stable")]
        heap = [(0, k) for k in range(ntiles)]
        fill = np.zeros(ntiles, np.int64)
        for node in order_t:
            load, k = heapq.heappop(heap)
            new_id[node] = base + k * P + fill[k]
            fill[k] += 1
            if fill[k] < P:
                heapq.heappush(heap, (load + int(indeg[node]), k))
        chunk_types += [t] * ntiles
        base += ntiles * P
    n_pad = base
    n_tiles = n_pad // P
    T = int(math.ceil(n_tiles / NCORES))
    n_grid = T * NCORES * P

    srcN = new_id[src]
    dstN = new_id[dst]

    order = np.argsort(dstN, kind="stable")
    ds, ss, es = dstN[order], srcN[order], et[order]
    tile_id = ds // P
    counts = np.bincount(tile_id, minlength=T * NCORES)
    B = int(math.ceil(counts.max() / P))
    starts = np.concatenate([[0], np.cumsum(counts)])
    pos = np.arange(E) - starts[tile_id]
    sp = pos % P
    sb = pos // P

    kvidx = np.zeros((T * NCORES, P, B), np.int32)
    codes = np.full((T * NCORES, P, B), 255.0, np.float16)
    codes5 = np.full((T * NCORES, B, P), 2000.0, np.float16)
    kvidx[tile_id, sp, sb] = ss.astype(np.int32)
    codes[tile_id, sp, sb] = (ds % P).astype(np.float16)
    codes5[tile_id, sb, sp] = (es * P + ds % P).astype(np.float16)
    codes5 = codes5.reshape(T * NCORES, B * P)

    import ml_dtypes
    x_perm = np.zeros((n_grid, D), np.float32)
    x_perm[new_id] = x
    xt_grid = np.ascontiguousarray(
        x_perm.reshape(n_grid // P, P, D).transpose(0, 2, 1)
    ).astype(ml_dtypes.bfloat16)

    # ---- fold mu and 1/sqrt(hd) into combined Q-side weights ----
    W_Q = np.asarray(W_Q, np.float32)
    W_K = np.asarray(W_K, np.float32)
    W_V = np.asarray(W_V, np.float32)
    W_edge = np.asarray(W_edge, np.float32)
    mu = np.asarray(mu, np.float32)
    wkv = np.zeros((NT, D, 2 * D), np.float32)
    wqe = np.zeros((NT, D, NET * D), np.float32)
    for t in range(NT):
        for h in range(H):
            sl = slice(h * HD, (h + 1) * HD)
            wkv[t, sl, sl] = W_K[t, h]
            wkv[t, sl, D + h * HD:D + (h + 1) * HD] = W_V[t, h]
            for e in range(NET):
                comb = (W_Q[t, h] @ W_edge[e, h].T) * (mu[h, e] / math.sqrt(HD))
                wqe[t, sl, e * D + h * HD:e * D + (h + 1) * HD] = comb
    wkv_bf = wkv.astype(ml_dtypes.bfloat16)
    wqe_bf = wqe.astype(ml_dtypes.bfloat16)
    # per-core, per-own-chunk Q-side weights (type-resolved on host so the
    # SPMD program stays core-independent)
    ct = np.asarray(chunk_types + [0] * (n_grid // P - len(chunk_types)),
                    np.int64)
    wqe_own_all = wqe_bf[ct]  # [n_grid//P, D, NET*D]

    gb = np.zeros((6, P, 4 * D), np.float32)
    gb[0, :, :D] = ln1_g
    gb[1, :, :D] = ln1_b
    gb[2, :, :D] = ln2_g
    gb[3, :, :D] = ln2_b
    gb[4, :, :D] = b_out
    gb[4, :, D:2 * D] = b2
    gb[5, :, :] = b1
    iota_f = np.tile(np.arange(P, dtype=np.float16)[None, :], (P, 1))
    iota5_f = (np.arange(P, dtype=np.float16)[:, None]
               + P * np.arange(NET, dtype=np.float16)[None, :])

    has_ln_gb1 = bool(np.any(ln1_g != 1) or np.any(ln1_b != 0))
    has_ln_gb2 = bool(np.any(ln2_g != 1) or np.any(ln2_b != 0))
    has_bout = bool(np.any(b_out != 0))
    has_b1 = bool(np.any(b1 != 0))
    has_b2 = bool(np.any(b2 != 0))

    key = (n_pad, T, B, has_ln_gb1, has_ln_gb2, has_bout, has_b1, has_b2,
           tuple(chunk_types))
    if key not in _CACHE:
        nc = bass.Bass("TRN2")
        _build(nc, chunk_types, n_pad, T, B,
               has_ln_gb1, has_ln_gb2, has_bout, has_b1, has_b2)
        # This walrus build rejects >1 sem-wait per instruction ("Too many
        # sync wait commands"); split excess waits onto InstEventSemaphore
        # carriers the same way Bacc.compile() does.
        bass_rust.generate_event_semaphores(nc)
        _CACHE[key] = nc
    nc = _CACHE[key]

    xt_tab = np.ascontiguousarray(xt_grid[:n_pad // P])
    in_maps = []
    for c in range(NCORES):
        t0, t1 = c * T, (c + 1) * T
        in_maps.append({
            "xt_bf": xt_tab,
            "xt_own": xt_grid[t0:t1],
            "x_own": x_perm[t0 * P:t1 * P],
            "wkv": wkv_bf,
            "wqe_own": wqe_own_all[c * T:(c + 1) * T],
            "wout": np.asarray(W_out, np.float32),
            "w1": np.asarray(W1, np.float32),
            "w2": np.asarray(W2, np.float32),
            "kvidx": kvidx[t0:t1],
            "codes": codes[t0:t1],
            "codes5": codes5[t0:t1],
            "iota_f": iota_f, "iota5_f": iota5_f, "gb": gb,
        })
    import os
    trace = bool(os.environ.get("KBENCH_TRACE"))
    res = run_bass_kernel_spmd(nc, in_maps, core_ids=list(range(NCORES)),
                               trace=trace)
    global LAST_RESULT
    LAST_RESULT = res
    out_new = np.concatenate([r["out"] for r in res.results], axis=0)
    return out_new[new_id].astype(np.float32)


LAST_RESULT = None

